# revision 14
# baseline (speedup 1.0000x reference)
"""Bilinear pooling kernel for 8 Trainium2 NeuronCores (Bass/Tile).

Computes out[b,n,v,o] = sum_{d,e} node[b,n,d] * veh[b,v,e] * W[o, d*E+e] + bias[o]
for B=16, N=64, V=16, D=E=128, O=256.

Strategy: tensor-shard over the output dim O (32 channels per core).
Two matmul stages pipelined in two o-halves:
  Stage A:  U[d, (o,b,v)] = sum_e W3[o,d,e] * veh[b,v,e]
            per o: lhsT = W3[o].T [e,d], rhs = vehT [e, (b,v)=256];
            psum groups of 4 channels, evacuated to bf16 U by vector+scalar.
  Stage B:  out[b][n, (o,v)] = sum_d node[b,n,d] * U[d, o-half, b, v]
            2 batches per psum tile via PE column tiling (the two 64-wide
            column groups stream concurrently), 4 batches per psum bank.
Inputs stream in 8 graduated chunks across the 3 DMA queues (sync/scalar
HWDGE + gpsimd SWDGE, ~80 GB/s each) ordered by consumption deadline.
Outputs are cast to bf16 and DMAd as contiguous 128KB tiles spread over
all 3 queues; the host adds the bias in f32 during the unshard.
"""

import sys

import numpy as np

sys.path.insert(0, "/opt/trn_rl_repo")

B, N, V = 16, 64, 16
D = 128
E = 128
O = 256
NCORES = 8
OS = O // NCORES  # 32 output channels per core

WARM = 6  # PE-clock warmup matmuls while the first input chunks stream in
TAIL_MM = 24  # post-compute dummy matmuls keep the PE clock high through
#               the NEFF's semaphore-reset epilogue (Tensor is its straggler)

_nc_cache = {}


def _build():
    from contextlib import ExitStack

    import concourse.tile as tile
    from concourse import bacc, mybir

    f32 = mybir.dt.float32
    bf16 = mybir.dt.bfloat16

    nc = bacc.Bacc("TRN2", target_bir_lowering=False)
    vehT_d = nc.dram_tensor("vehT", [E, B * V], bf16, kind="ExternalInput")
    # w chunks: g0 split in two (earliest-needed), rest 4 channels each
    wg_d = {}
    for name, nch in (("w0a", 2), ("w0b", 2), ("w1", 4), ("w2", 4), ("w3", 4),
                      ("w4", 4), ("w5", 4), ("w6", 4), ("w7", 4)):
        wg_d[name] = nc.dram_tensor(name, [E, nch * D], bf16, kind="ExternalInput")
    nodeT_d = nc.dram_tensor("nodeT", [D, B * N], bf16, kind="ExternalInput")
    # out tiles: t = h*4+q covers batches 4q..4q+3 of o-half h;
    # [128, 512] per tile: partition p=(b%2)*64+n, free = (j=(b%4)//2, ch, v)
    out_d = nc.dram_tensor("out", [8, 128, 512], bf16, kind="ExternalOutput")

    with ExitStack() as ctx:
        tc = ctx.enter_context(tile.TileContext(nc))
        const = ctx.enter_context(tc.tile_pool(name="const", bufs=1))
        upool = ctx.enter_context(tc.tile_pool(name="u", bufs=2))
        psA = ctx.enter_context(tc.tile_pool(name="psA", bufs=2, space="PSUM"))
        psB = ctx.enter_context(tc.tile_pool(name="psB", bufs=4, space="PSUM"))
        outp = ctx.enter_context(tc.tile_pool(name="outp", bufs=4))

        # ---- input DMAs: 3 parallel queues, chunks ordered by deadline ----
        # HWDGE (sync/scalar) run ~80 GB/s; gpsimd SWDGE only ~40 GB/s so it
        # carries just nodeT (not needed until stage B0).
        wt = {}

        def in_dma(eng, name, cols):
            t = const.tile([E, cols], bf16, name=f"{name}t")
            eng.dma_start(t[:], wg_d[name][:])
            wt[name] = t
            return t

        vehT_t = const.tile([E, B * V], bf16)
        nc.scalar.dma_start(vehT_t[:], vehT_d[:])  # stage A rhs, needed first
        for name in ("w0a", "w0b"):  # sync queue: g0 halves first
            in_dma(nc.sync, name, 2 * D)
        nodeT_t = const.tile([D, B * N], bf16)
        nc.gpsimd.dma_start(nodeT_t[:], nodeT_d[:])  # needed at stage B0
        for name in ("w1", "w3", "w5", "w7"):
            in_dma(nc.scalar, name, 4 * D)
        for name in ("w2", "w4", "w6"):
            in_dma(nc.sync, name, 4 * D)
        nodeT = nodeT_t[:]
        vehT = vehT_t[:]

        def wsel(o):
            # lhsT for stage-A channel o (core-local 0..31)
            if o < 2:
                return wt["w0a"][:, o * D : (o + 1) * D]
            if o < 4:
                return wt["w0b"][:, (o - 2) * D : (o - 1) * D]
            t = wt[f"w{o // 4}"]
            return t[:, (o % 4) * D : (o % 4 + 1) * D]

        # ---- PE warmup on a zeroed tile (vector memset is its first op) ----
        warm = const.tile([E, B * V], bf16)
        nc.vector.memset(warm[:], 0)
        wps = psA.tile([D, 4, B * V], f32, tag="pa")
        for i in range(WARM):
            nc.tensor.matmul(wps[:, i % 4], warm[:, 0:D], warm[:], start=True, stop=True)

        U = [
            upool.tile([D, 16, B, V], bf16, tag="U", name=f"U{h}") for h in range(2)
        ]

        def stageA(g):
            # 4 channels (o = 4g .. 4g+3) -> psum [128, 4, 256] -> U[h]
            pa = psA.tile([D, 4, B * V], f32, tag="pa")
            for i in range(4):
                nc.tensor.matmul(pa[:, i], wsel(4 * g + i), vehT, start=True, stop=True)
            h, gl = divmod(g, 4)
            dst = U[h]
            nc.vector.tensor_copy(dst[:, 4 * gl : 4 * gl + 2, :, :], pa[:, 0:2])
            nc.scalar.copy(dst[:, 4 * gl + 2 : 4 * gl + 4, :, :], pa[:, 2:4])

        def stageB(h, q, ceng, deng, split=False):
            # batches 4q..4q+3 of o-half h -> psum [128, 2, 256] -> out tile
            pb = psB.tile([N * 2, 2, 256], f32, tag="pb")
            for j in range(2):
                for pbi in range(2):
                    b = 4 * q + 2 * j + pbi
                    nc.tensor.matmul(
                        pb[64 * pbi : 64 * (pbi + 1), j],
                        nodeT[:, b * N : (b + 1) * N],
                        U[h][:, :, b, :],
                        start=True,
                        stop=True,
                    )
            ob = outp.tile([128, 512], bf16)
            if not split:
                ceng(ob[:], pb[:])
                deng.dma_start(out_d[4 * h + q], ob[:])
            else:
                # final tile: halve copy+DMA latency via two queues
                nc.vector.tensor_copy(ob[:, 0:256], pb[:, 0])
                nc.scalar.copy(ob[:, 256:512], pb[:, 1])
                nc.sync.dma_start(out_d[4 * h + q, :, 0:256], ob[:, 0:256])
                nc.scalar.dma_start(out_d[4 * h + q, :, 256:512], ob[:, 256:512])
            return ob

        vcp = nc.vector.tensor_copy
        scp = nc.scalar.copy
        # pipeline: A g0..g4 (g4 hides the U-copy latency before B0),
        # B(h0), A g5..g7, B(h1)
        for g in range(5):
            stageA(g)
        stageB(0, 0, vcp, nc.sync)
        stageB(0, 1, scp, nc.gpsimd)
        stageB(0, 2, vcp, nc.scalar)
        stageB(0, 3, scp, nc.sync)
        for g in range(5, 8):
            stageA(g)
        stageB(1, 0, vcp, nc.scalar)
        stageB(1, 1, scp, nc.gpsimd)
        stageB(1, 2, vcp, nc.sync)
        ob_last = stageB(1, 3, None, None, split=True)

        # keep the PE clock high through the NEFF's semaphore-reset epilogue;
        # reading ob_last pins these after the final output copy (the tile
        # scheduler hoists dependency-free work earlier)
        tps = psA.tile([D, 4, B * V], f32, tag="pa")
        for i in range(TAIL_MM):
            nc.tensor.matmul(
                tps[:, i % 4], ob_last[:, 0:D], warm[:], start=True, stop=True
            )

    nc.compile()
    return nc


def _get_nc():
    if "nc" not in _nc_cache:
        _nc_cache["nc"] = _build()
    return _nc_cache["nc"]


def _prep_inputs(node_embed, veh_fea, W, b):
    import ml_dtypes

    def cast(x):
        return np.ascontiguousarray(x.astype(ml_dtypes.bfloat16))

    node_embed = np.asarray(node_embed, dtype=np.float32)
    veh_fea = np.asarray(veh_fea, dtype=np.float32)
    W = np.asarray(W, dtype=np.float32)

    nodeT = cast(node_embed.transpose(2, 0, 1).reshape(D, B * N))
    vehT = cast(veh_fea.transpose(2, 0, 1).reshape(E, B * V))
    W3 = W.reshape(O, D, E)

    in_maps = []
    for c in range(NCORES):
        # [E, o_local, D] channel-major weights for this core's O-shard
        wtc = W3[c * OS : (c + 1) * OS].transpose(2, 0, 1).reshape(E, OS * D)
        m = {"vehT": vehT, "nodeT": nodeT}
        m["w0a"] = cast(wtc[:, 0 : 2 * D])
        m["w0b"] = cast(wtc[:, 2 * D : 4 * D])
        for g in range(1, 8):
            m[f"w{g}"] = cast(wtc[:, g * 4 * D : (g + 1) * 4 * D])
        in_maps.append(m)
    return in_maps


def run(node_embed, veh_fea, W, b, trace=False):
    from concourse.bass_utils import run_bass_kernel_spmd

    nc = _get_nc()
    in_maps = _prep_inputs(node_embed, veh_fea, W, b)
    res = run_bass_kernel_spmd(nc, in_maps, list(range(NCORES)), trace=trace)
    outs = []
    for r in res.results:
        # [8, 128, 512] -> [h, q, pb, n, j, ch, v] -> [b, n, v, (h,ch)]
        arr = np.asarray(r["out"]).astype(np.float32)
        arr = arr.reshape(2, 4, 2, 64, 2, 16, 16)
        arr = arr.transpose(1, 4, 2, 3, 6, 0, 5).reshape(B, N, V, OS)
        outs.append(arr)
    full = np.concatenate(outs, axis=3) + np.asarray(b, np.float32)
    return np.ascontiguousarray(full, dtype=np.float32), res


def kernel(node_embed, veh_fea, W, b):
    return run(node_embed, veh_fea, W, b)[0]


# revision 20
# speedup vs baseline: 1.1278x; 1.1278x over previous
"""Bilinear pooling kernel for 8 Trainium2 NeuronCores (Bass/Tile).

Computes out[b,n,v,o] = sum_{d,e} node[b,n,d] * veh[b,v,e] * W[o, d*E+e] + bias[o]
for B=16, N=64, V=16, D=E=128, O=256.

Strategy: tensor-shard over the output dim O (32 channels per core).
Two matmul stages pipelined in two o-halves:
  Stage A:  U[d, (o,b,v)] = sum_e W3[o,d,e] * veh[b,v,e]
            per o: lhsT = W3[o].T [e,d], rhs = vehT [e, (b,v)=256];
            psum groups of 4 channels, evacuated to bf16 U by vector+scalar.
  Stage B:  out[b][n, (o,v)] = sum_d node[b,n,d] * U[d, o-half, b, v]
            2 batches per psum tile via PE column tiling (the two 64-wide
            column groups stream concurrently), 4 batches per psum bank.
Inputs stream in 8 graduated chunks across the 3 DMA queues (sync/scalar
HWDGE + gpsimd SWDGE, ~80 GB/s each) ordered by consumption deadline.
Outputs are cast to bf16 and DMAd as contiguous 128KB tiles spread over
all 3 queues; the host adds the bias in f32 during the unshard.
"""

import sys

import numpy as np

sys.path.insert(0, "/opt/trn_rl_repo")

B, N, V = 16, 64, 16
D = 128
E = 128
O = 256
NCORES = 8
OS = O // NCORES  # 32 output channels per core

WARM = 14  # PE-clock warmup matmuls while the first input chunk streams in

_nc_cache = {}


def _build():
    from contextlib import ExitStack

    import concourse.tile as tile
    from concourse import bacc, mybir

    f32 = mybir.dt.float32
    bf16 = mybir.dt.bfloat16

    nc = bacc.Bacc("TRN2", target_bir_lowering=False)
    # DMA queues pay a fixed multi-us startup + run faster with wider rows,
    # so inputs come as 4 wide-row chunks (not many small ones):
    #   sync:   K1 = [vehT | w ch0-7]   (2.5KB rows), then K2 = [w ch24-31]
    #   scalar: K3 = [w ch8-23]         (4KB rows)
    #   gpsimd: nodeT                   (2KB rows)
    k1_d = nc.dram_tensor("k1", [E, B * V + 8 * D], bf16, kind="ExternalInput")
    k2_d = nc.dram_tensor("k2", [E, 8 * D], bf16, kind="ExternalInput")
    k3_d = nc.dram_tensor("k3", [E, 16 * D], bf16, kind="ExternalInput")
    nodeT_d = nc.dram_tensor("nodeT", [D, B * N], bf16, kind="ExternalInput")
    # out: one tile per (o-half, q-tile pair): [128, 2, 512] per tile,
    # partition p=(b%2)*64+n, free = (qpair, j=(b%4)//2, ch, v)
    out_d = nc.dram_tensor("out", [4, 128, 1024], bf16, kind="ExternalOutput")

    with ExitStack() as ctx:
        tc = ctx.enter_context(tile.TileContext(nc))
        const = ctx.enter_context(tc.tile_pool(name="const", bufs=1))
        upool = ctx.enter_context(tc.tile_pool(name="u", bufs=2))
        psA = ctx.enter_context(tc.tile_pool(name="psA", bufs=2, space="PSUM"))
        psB = ctx.enter_context(tc.tile_pool(name="psB", bufs=4, space="PSUM"))
        outp = ctx.enter_context(tc.tile_pool(name="outp", bufs=4))

        # ---- input DMAs ----
        k1 = const.tile([E, B * V + 8 * D], bf16)
        nc.sync.dma_start(k1[:], k1_d[:])
        k3 = const.tile([E, 16 * D], bf16)
        nc.scalar.dma_start(k3[:], k3_d[:])
        nodeT_t = const.tile([D, B * N], bf16)
        nc.gpsimd.dma_start(nodeT_t[:], nodeT_d[:])  # needed at stage B0
        k2 = const.tile([E, 8 * D], bf16)
        nc.sync.dma_start(k2[:], k2_d[:])
        nodeT = nodeT_t[:]
        vehT = k1[:, : B * V]

        def wsel(o):
            # lhsT for stage-A channel o (core-local 0..31)
            if o < 8:
                return k1[:, B * V + o * D : B * V + (o + 1) * D]
            if o < 24:
                return k3[:, (o - 8) * D : (o - 7) * D]
            return k2[:, (o - 24) * D : (o - 23) * D]

        # ---- PE warmup on a zeroed tile (vector memset is its first op) ----
        warm = const.tile([E, B * V], bf16)
        nc.vector.memset(warm[:], 0)
        wps = psA.tile([D, 4, B * V], f32, tag="pa")
        for i in range(WARM):
            nc.tensor.matmul(wps[:, i % 4], warm[:, 0:D], warm[:], start=True, stop=True)

        U = [
            upool.tile([D, 16, B, V], bf16, tag="U", name=f"U{h}") for h in range(2)
        ]

        def stageA(g):
            # 4 channels (o = 4g .. 4g+3) -> psum [128, 4, 256] -> U[h]
            pa = psA.tile([D, 4, B * V], f32, tag="pa")
            for i in range(4):
                nc.tensor.matmul(pa[:, i], wsel(4 * g + i), vehT, start=True, stop=True)
            h, gl = divmod(g, 4)
            dst = U[h]
            nc.vector.tensor_copy(dst[:, 4 * gl : 4 * gl + 2, :, :], pa[:, 0:2])
            nc.scalar.copy(dst[:, 4 * gl + 2 : 4 * gl + 4, :, :], pa[:, 2:4])

        def stageB(h, q, ob, ceng, deng):
            # batches 4q..4q+3 of o-half h -> psum [128, 2, 256] -> ob slot;
            # after the odd q of each pair, DMA the 256KB staging tile
            # (2KB rows run the queues at full tilt)
            pb = psB.tile([N * 2, 2, 256], f32, tag="pb")
            for j in range(2):
                for pbi in range(2):
                    b = 4 * q + 2 * j + pbi
                    nc.tensor.matmul(
                        pb[64 * pbi : 64 * (pbi + 1), j],
                        nodeT[:, b * N : (b + 1) * N],
                        U[h][:, :, b, :],
                        start=True,
                        stop=True,
                    )
            ceng(ob[:, q % 2], pb[:])
            if q % 2 == 1:
                deng.dma_start(out_d[2 * h + q // 2], ob[:])

        vcp = nc.vector.tensor_copy
        scp = nc.scalar.copy
        # pipeline: A g0..g4 (g4 hides the U-copy latency before B0),
        # B(h0), A g5..g7, B(h1)
        for g in range(5):
            stageA(g)
        ob0 = outp.tile([128, 2, 512], bf16, name="ob0")
        ob1 = outp.tile([128, 2, 512], bf16, name="ob1")
        stageB(0, 0, ob0, vcp, None)
        stageB(0, 1, ob0, scp, nc.sync)
        stageB(0, 2, ob1, vcp, None)
        stageB(0, 3, ob1, scp, nc.scalar)
        for g in range(5, 8):
            stageA(g)
        ob2 = outp.tile([128, 2, 512], bf16, name="ob2")
        ob3 = outp.tile([128, 2, 512], bf16, name="ob3")
        stageB(1, 0, ob2, vcp, None)
        stageB(1, 1, ob2, scp, nc.sync)
        stageB(1, 2, ob3, vcp, None)
        stageB(1, 3, ob3, scp, nc.scalar)

    nc.compile()
    return nc


def _get_nc():
    if "nc" not in _nc_cache:
        _nc_cache["nc"] = _build()
    return _nc_cache["nc"]


def _prep_inputs(node_embed, veh_fea, W, b):
    import ml_dtypes

    def cast(x):
        return np.ascontiguousarray(x.astype(ml_dtypes.bfloat16))

    node_embed = np.asarray(node_embed, dtype=np.float32)
    veh_fea = np.asarray(veh_fea, dtype=np.float32)
    W = np.asarray(W, dtype=np.float32)

    nodeT = cast(node_embed.transpose(2, 0, 1).reshape(D, B * N))
    vehT = cast(veh_fea.transpose(2, 0, 1).reshape(E, B * V))
    W3 = W.reshape(O, D, E)

    in_maps = []
    for c in range(NCORES):
        # [E, o_local, D] channel-major weights for this core's O-shard
        wtc = W3[c * OS : (c + 1) * OS].transpose(2, 0, 1).reshape(E, OS * D)
        in_maps.append(
            {
                "k1": cast(np.concatenate([vehT, wtc[:, 0 : 8 * D]], axis=1)),
                "k3": cast(wtc[:, 8 * D : 24 * D]),
                "k2": cast(wtc[:, 24 * D : 32 * D]),
                "nodeT": nodeT,
            }
        )
    return in_maps


def run(node_embed, veh_fea, W, b, trace=False):
    from concourse.bass_utils import run_bass_kernel_spmd

    nc = _get_nc()
    in_maps = _prep_inputs(node_embed, veh_fea, W, b)
    res = run_bass_kernel_spmd(nc, in_maps, list(range(NCORES)), trace=trace)
    outs = []
    for r in res.results:
        # [4, 128, 1024] -> [h, qp, pb, n, j2, j, ch, v] -> [b, n, v, (h,ch)]
        # with b = 8*qp + 4*j2 + 2*j + pb
        arr = np.asarray(r["out"]).astype(np.float32)
        arr = arr.reshape(2, 2, 2, 64, 2, 2, 16, 16)
        arr = arr.transpose(1, 4, 5, 2, 3, 7, 0, 6).reshape(B, N, V, OS)
        outs.append(arr)
    full = np.concatenate(outs, axis=3) + np.asarray(b, np.float32)
    return np.ascontiguousarray(full, dtype=np.float32), res


def kernel(node_embed, veh_fea, W, b):
    return run(node_embed, veh_fea, W, b)[0]


# revision 25
# speedup vs baseline: 1.2080x; 1.0711x over previous
"""Bilinear pooling kernel for 8 Trainium2 NeuronCores (Bass/Tile).

Computes out[b,n,v,o] = sum_{d,e} node[b,n,d] * veh[b,v,e] * W[o, d*E+e] + bias[o]
for B=16, N=64, V=16, D=E=128, O=256.

Strategy: tensor-shard over the output dim O (32 channels per core).
Two matmul stages pipelined in two o-halves:
  Stage A:  U[d, (o,b,v)] = sum_e W3[o,d,e] * veh[b,v,e]
            per o: lhsT = W3[o].T [e,d], rhs = vehT [e, (b,v)=256];
            psum groups of 4 channels, evacuated to bf16 U by vector+scalar.
  Stage B:  out[b][n, (o,v)] = sum_d node[b,n,d] * U[d, o-half, b, v]
            2 batches per psum tile via PE column tiling (the two 64-wide
            column groups stream concurrently), 4 batches per psum bank.
Inputs stream in 8 graduated chunks across the 3 DMA queues (sync/scalar
HWDGE + gpsimd SWDGE, ~80 GB/s each) ordered by consumption deadline.
Outputs are cast to bf16 and DMAd as contiguous 128KB tiles spread over
all 3 queues; the host adds the bias in f32 during the unshard.
"""

import sys

import numpy as np

sys.path.insert(0, "/opt/trn_rl_repo")

B, N, V = 16, 64, 16
D = 128
E = 128
O = 256
NCORES = 8
OS = O // NCORES  # 32 output channels per core

WARM = 6  # PE-clock warmup matmuls while the first input chunk streams in

_nc_cache = {}


def _build():
    from contextlib import ExitStack

    import concourse.tile as tile
    from concourse import bacc, mybir

    f32 = mybir.dt.float32
    bf16 = mybir.dt.bfloat16

    nc = bacc.Bacc("TRN2", target_bir_lowering=False)
    # DMA queues pay a fixed multi-us startup + run faster with wider rows,
    # so inputs come as 4 wide-row chunks (not many small ones):
    #   sync:   K1 = [vehT | w ch0-7]   (2.5KB rows), then K2 = [w ch24-31]
    #   scalar: K3 = [w ch8-23]         (4KB rows)
    #   gpsimd: nodeT                   (2KB rows)
    k1_d = nc.dram_tensor("k1", [E, B * V + 8 * D], bf16, kind="ExternalInput")
    k2_d = nc.dram_tensor("k2", [E, 8 * D], bf16, kind="ExternalInput")
    k3_d = nc.dram_tensor("k3", [E, 16 * D], bf16, kind="ExternalInput")
    nodeT_d = nc.dram_tensor("nodeT", [D, B * N], bf16, kind="ExternalInput")
    # out: one tile per (o-half, q-tile pair): [128, 2, 512] per tile,
    # partition p=(b%2)*64+n, free = (qpair, j=(b%4)//2, ch, v)
    out_d = nc.dram_tensor("out", [4, 128, 1024], bf16, kind="ExternalOutput")

    with ExitStack() as ctx:
        tc = ctx.enter_context(tile.TileContext(nc))
        const = ctx.enter_context(tc.tile_pool(name="const", bufs=1))
        upool = ctx.enter_context(tc.tile_pool(name="u", bufs=2))
        psA = ctx.enter_context(tc.tile_pool(name="psA", bufs=3, space="PSUM"))
        psB = ctx.enter_context(tc.tile_pool(name="psB", bufs=2, space="PSUM"))
        outp = ctx.enter_context(tc.tile_pool(name="outp", bufs=4))

        # ---- input DMAs ----
        k1 = const.tile([E, B * V + 8 * D], bf16)
        nc.sync.dma_start(k1[:], k1_d[:])
        k3 = const.tile([E, 16 * D], bf16)
        nc.scalar.dma_start(k3[:], k3_d[:])
        nodeT_t = const.tile([D, B * N], bf16)
        nc.gpsimd.dma_start(nodeT_t[:], nodeT_d[:])  # needed at stage B0
        k2 = const.tile([E, 8 * D], bf16)
        nc.sync.dma_start(k2[:], k2_d[:])
        nodeT = nodeT_t[:]
        vehT = k1[:, : B * V]

        def wsel(o):
            # lhsT for stage-A channel o (core-local 0..31)
            if o < 8:
                return k1[:, B * V + o * D : B * V + (o + 1) * D]
            if o < 24:
                return k3[:, (o - 8) * D : (o - 7) * D]
            return k2[:, (o - 24) * D : (o - 23) * D]

        # ---- PE warmup on a zeroed tile (vector memset is its first op) ----
        warm = const.tile([E, B * V], bf16)
        nc.vector.memset(warm[:], 0)
        # 1-element scalar ACTIVATE so the act-table load (1.3us) happens
        # during the input-DMA wait, not before the first real psum copy
        # (self-copy of an uninitialized tile: no deps, result unused)
        actwarm = const.tile([E, 2], bf16)
        nc.scalar.copy(actwarm[:, 0:1], actwarm[:, 1:2])
        wps = psA.tile([D, 4, B * V], f32, tag="pa")
        for i in range(WARM):
            nc.tensor.matmul(wps[:, i % 4], warm[:, 0:D], warm[:], start=True, stop=True)

        U = [
            upool.tile([D, 16, B, V], bf16, tag="U", name=f"U{h}") for h in range(2)
        ]

        def stageA(g):
            # 4 channels (o = 4g .. 4g+3) -> psum [128, 4, 256] -> U[h]
            pa = psA.tile([D, 4, B * V], f32, tag="pa")
            for i in range(4):
                nc.tensor.matmul(pa[:, i], wsel(4 * g + i), vehT, start=True, stop=True)
            h, gl = divmod(g, 4)
            dst = U[h]
            nc.vector.tensor_copy(dst[:, 4 * gl : 4 * gl + 2, :, :], pa[:, 0:2])
            nc.scalar.copy(dst[:, 4 * gl + 2 : 4 * gl + 4, :, :], pa[:, 2:4])

        def stageB(h, q, ob, ceng, deng):
            # batches 4q..4q+3 of o-half h -> psum [128, 2, 256] -> ob slot;
            # after the odd q of each pair, DMA the 256KB staging tile
            # (2KB rows run the queues at full tilt)
            pb = psB.tile([N * 2, 2, 256], f32, tag="pb")
            for j in range(2):
                for pbi in range(2):
                    b = 4 * q + 2 * j + pbi
                    nc.tensor.matmul(
                        pb[64 * pbi : 64 * (pbi + 1), j],
                        nodeT[:, b * N : (b + 1) * N],
                        U[h][:, :, b, :],
                        start=True,
                        stop=True,
                    )
            ceng(ob[:, q % 2], pb[:])
            if q % 2 == 1:
                deng.dma_start(out_d[2 * h + q // 2], ob[:])

        vcp = nc.vector.tensor_copy
        scp = nc.scalar.copy
        # pipeline: A g0..g4 (g4 hides the U-copy latency before B0), then
        # B(h0) with A g5-g7 interleaved so U[h1] copies start early, B(h1)
        for g in range(5):
            stageA(g)
        ob0 = outp.tile([128, 2, 512], bf16, name="ob0")
        ob1 = outp.tile([128, 2, 512], bf16, name="ob1")
        stageB(0, 0, ob0, vcp, None)
        stageB(0, 1, ob0, scp, nc.sync)
        stageA(5)
        stageB(0, 2, ob1, vcp, None)
        stageB(0, 3, ob1, scp, nc.scalar)
        stageA(6)
        stageA(7)
        ob2 = outp.tile([128, 2, 512], bf16, name="ob2")
        ob3 = outp.tile([128, 2, 512], bf16, name="ob3")
        stageB(1, 0, ob2, vcp, None)
        stageB(1, 1, ob2, scp, nc.sync)
        stageB(1, 2, ob3, vcp, None)
        stageB(1, 3, ob3, scp, nc.scalar)

    nc.compile()
    return nc


def _get_nc():
    if "nc" not in _nc_cache:
        _nc_cache["nc"] = _build()
    return _nc_cache["nc"]


def _prep_inputs(node_embed, veh_fea, W, b):
    import ml_dtypes

    def cast(x):
        return np.ascontiguousarray(x.astype(ml_dtypes.bfloat16))

    node_embed = np.asarray(node_embed, dtype=np.float32)
    veh_fea = np.asarray(veh_fea, dtype=np.float32)
    W = np.asarray(W, dtype=np.float32)

    nodeT = cast(node_embed.transpose(2, 0, 1).reshape(D, B * N))
    vehT = cast(veh_fea.transpose(2, 0, 1).reshape(E, B * V))
    W3 = W.reshape(O, D, E)

    in_maps = []
    for c in range(NCORES):
        # [E, o_local, D] channel-major weights for this core's O-shard
        wtc = W3[c * OS : (c + 1) * OS].transpose(2, 0, 1).reshape(E, OS * D)
        in_maps.append(
            {
                "k1": cast(np.concatenate([vehT, wtc[:, 0 : 8 * D]], axis=1)),
                "k3": cast(wtc[:, 8 * D : 24 * D]),
                "k2": cast(wtc[:, 24 * D : 32 * D]),
                "nodeT": nodeT,
            }
        )
    return in_maps


def run(node_embed, veh_fea, W, b, trace=False):
    from concourse.bass_utils import run_bass_kernel_spmd

    nc = _get_nc()
    in_maps = _prep_inputs(node_embed, veh_fea, W, b)
    res = run_bass_kernel_spmd(nc, in_maps, list(range(NCORES)), trace=trace)
    outs = []
    for r in res.results:
        # [4, 128, 1024] -> [h, qp, pb, n, j2, j, ch, v] -> [b, n, v, (h,ch)]
        # with b = 8*qp + 4*j2 + 2*j + pb
        arr = np.asarray(r["out"]).astype(np.float32)
        arr = arr.reshape(2, 2, 2, 64, 2, 2, 16, 16)
        arr = arr.transpose(1, 4, 5, 2, 3, 7, 0, 6).reshape(B, N, V, OS)
        outs.append(arr)
    full = np.concatenate(outs, axis=3) + np.asarray(b, np.float32)
    return np.ascontiguousarray(full, dtype=np.float32), res


def kernel(node_embed, veh_fea, W, b):
    return run(node_embed, veh_fea, W, b)[0]
